# revision 9
# baseline (speedup 1.0000x reference)
"""TRN2 Bass kernel for nn_BottleneckAdapter: projection -> down -> LayerNorm ->
SwiGLU -> up, data-parallel over 8 NeuronCores; the final residual add runs on
host (saves 16 MB/core of DMA, the device is PE-bound).

Shapes (hardcoded): B=16, S=2048, C=768, Q=1024, D=64. Tokens = B*S = 32768,
4096 per core. All matmuls run in float32r (TF32-class, full PE rate).

Math folding (host-side, exact fp32 algebra):
 - mean-centering of LayerNorm folded into Wdown:  Wdc = Wdown - colmean(Wdown)
   => the down matmul directly yields c = down - mean_d(down).
 - gamma folded into Wl1/Wl2 columns; beta and bl1/bl2 folded into an extra
   ones-row (K=65) bias row => normed_pre = c * rstd is all the kernel needs.

Schedule: per token macro-tile (512 tokens for the first six, 256 for the
last four so the tail latency chains are short), three stages emitted with a
software pipeline skew so PE never waits on the ACT/DVE LayerNorm chain:
  A(t): load X^T tile, mm1 (8q x 6c MM) + psum->sbuf copies, mm2 (8 MM)
  B(t): LN (square/var-mm/sqrt/recip/mult) + o1/gate MM + silu + gate mult
  C(t): mm5 (2 MM per 128-token subtile) + psum->sbuf copy + store
emission order: A0 A1 B0 A2 B1 C0 A3 B2 C1 ...  PSUM->SBUF copies are split
between DVE and ACT to balance engine load; output stores alternate between
the HWDGE (sync) and SWDGE (gpsimd) queues.
"""
import sys
import os

sys.path.insert(0, "/opt/trn_rl_repo")

import numpy as np

import concourse.bass as bass
import concourse.mybir as mybir
import concourse.tile as tile
from concourse import bacc
from concourse import bass_utils

F32 = mybir.dt.float32
F32R = mybir.dt.float32r

NCORES = 8
B, S, C, Q, D = 16, 2048, 768, 1024, 64
TOK = B * S                 # 32768
TPC = TOK // NCORES         # 4096 tokens per core
CS = C // 128               # 6 c-subtiles
QS = Q // 128               # 8 q-subtiles
EPS = 1e-5

# macro-tile layout: (t0, width)
TILES = [(i * 512, 512) for i in range(6)] + \
        [(3072 + i * 256, 256) for i in range(4)]

_CACHE = {}


def _build():
    nc = bacc.Bacc("TRN2", target_bir_lowering=False, debug=False,
                   enable_asserts=True, num_devices=NCORES)
    xt = nc.dram_tensor("xt", [C, TPC], F32R, kind="ExternalInput").ap()
    wprojT = nc.dram_tensor("wprojT", [C, Q], F32R, kind="ExternalInput").ap()
    wdc = nc.dram_tensor("wdc", [Q, D], F32R, kind="ExternalInput").ap()
    ones64 = nc.dram_tensor("ones64", [D, D], F32R, kind="ExternalInput").ap()
    w1aug = nc.dram_tensor("w1aug", [D + 1, D], F32R, kind="ExternalInput").ap()
    w2aug = nc.dram_tensor("w2aug", [D + 1, D], F32R, kind="ExternalInput").ap()
    wupT = nc.dram_tensor("wupT", [D, Q], F32R, kind="ExternalInput").ap()
    out = nc.dram_tensor("out", [TPC, Q], F32, kind="ExternalOutput").ap()

    xt_r = xt.rearrange("(o p) t -> p o t", p=128)          # [128, CS, TPC]
    wp_r = wprojT.rearrange("(o p) q -> p o q", p=128)      # [128, CS, Q]
    wd_r = wdc.rearrange("(o p) d -> p o d", p=128)         # [128, QS, D]

    with tile.TileContext(nc) as tc:
        with tc.tile_pool(name="wres", bufs=1) as wres, \
             tc.tile_pool(name="xp", bufs=2) as xp, \
             tc.tile_pool(name="pp", bufs=2) as pp, \
             tc.tile_pool(name="sm", bufs=2) as sm, \
             tc.tile_pool(name="op", bufs=4) as op, \
             tc.tile_pool(name="ps1", bufs=2, space="PSUM") as ps1, \
             tc.tile_pool(name="ps2", bufs=2, space="PSUM") as ps2, \
             tc.tile_pool(name="pss", bufs=2, space="PSUM") as pss, \
             tc.tile_pool(name="ps5", bufs=2, space="PSUM") as ps5:

            wp = wres.tile([128, CS, Q], F32R)
            wd = wres.tile([128, QS, D], F32R)
            on64 = wres.tile([D, D], F32R)
            w1 = wres.tile([D + 1, D], F32R)
            w2 = wres.tile([D + 1, D], F32R)
            wu = wres.tile([D, Q], F32R)
            epst = wres.tile([D, 1], F32)

            state = {}
            store_q = [0]   # alternate stores between sync/gpsimd queues

            def stage_a(k, first=0):
                t0, W = TILES[k]
                xtile = xp.tile([128, CS, W], F32R, tag="xtile")
                for c in range(CS):
                    nc.sync.dma_start(xtile[:, c, :], xt_r[:, c, t0:t0 + W])
                    if first == 1:
                        # interleave weight-block loads with the first x tile
                        nc.sync.dma_start(wp[:, c, :], wp_r[:, c, :])
                if first == 1:
                    nc.sync.dma_start(wd[:], wd_r[:])
                    nc.gpsimd.memset(epst[:], EPS)
                elif first == 2:
                    # weights not needed until B0/C0 load after x tile 1
                    nc.sync.dma_start(on64[:], ones64[:])
                    nc.sync.dma_start(w1[:], w1aug[:])
                    nc.sync.dma_start(w2[:], w2aug[:])
                    nc.sync.dma_start(wu[:], wupT[:])
                ptile = pp.tile([128, QS, W], F32R, tag="ptile")
                for q in range(QS):
                    p1 = ps1.tile([128, W], F32, tag="p1")
                    for c in range(CS):
                        nc.tensor.matmul(p1[:], wp[:, c, q * 128:(q + 1) * 128],
                                         xtile[:, c, :],
                                         start=(c == 0), stop=(c == CS - 1))
                    if q % 3 == 2:      # 3 of 8 copies on ACT (Copy is in
                        nc.scalar.copy(ptile[:, q, :], p1[:])   # every table)
                    else:
                        nc.vector.tensor_copy(ptile[:, q, :], p1[:])
                p2 = ps2.tile([D, W], F32, tag="p2")
                for q in range(QS):
                    nc.tensor.matmul(p2[:], wd[:, q, :], ptile[:, q, :],
                                     start=(q == 0), stop=(q == QS - 1))
                state[("p2", k)] = p2

            def stage_b(k):
                t0, W = TILES[k]
                p2 = state.pop(("p2", k))
                csq = sm.tile([D, W], F32R, tag="csq")
                nc.scalar.activation(csq[:], p2[:],
                                     mybir.ActivationFunctionType.Square)
                varp = pss.tile([D, W], F32, tag="small")
                nc.tensor.matmul(varp[:], on64[:], csq[:], start=True, stop=True)
                s = sm.tile([D, W], F32, tag="s")
                nc.scalar.activation(s[:], varp[:],
                                     mybir.ActivationFunctionType.Sqrt,
                                     bias=epst[:])
                rstd = sm.tile([D, W], F32, tag="rstd")
                nc.vector.reciprocal(rstd[:], s[:])
                normed = sm.tile([D + 1, W], F32R, tag="normed")
                nc.vector.tensor_mul(normed[0:D, :], p2[:], rstd[:])
                nc.gpsimd.memset(normed[D:D + 1, :].bitcast(F32), 1.0)
                o1p = pss.tile([D, W], F32, tag="small")
                nc.tensor.matmul(o1p[:], w1[:], normed[:], start=True, stop=True)
                gatep = pss.tile([D, W], F32, tag="small")
                nc.tensor.matmul(gatep[:], w2[:], normed[:], start=True, stop=True)
                swish = sm.tile([D, W], F32, tag="swish")
                nc.scalar.activation(swish[:], o1p[:],
                                     mybir.ActivationFunctionType.Silu)
                actT = sm.tile([D, W], F32R, tag="actT")
                nc.vector.tensor_mul(actT[:], gatep[:], swish[:])
                state[("actT", k)] = actT

            def stage_c(k):
                t0, W = TILES[k]
                actT = state.pop(("actT", k))
                for ts in range(W // 128):
                    r0 = t0 + ts * 128
                    otile = op.tile([128, Q], F32, tag="otile")
                    for qh in range(2):
                        p5 = ps5.tile([128, 512], F32, tag="p5")
                        nc.tensor.matmul(p5[:],
                                         actT[:, ts * 128:(ts + 1) * 128],
                                         wu[:, qh * 512:(qh + 1) * 512],
                                         start=True, stop=True)
                        if qh == 1:
                            nc.scalar.copy(otile[:, qh * 512:(qh + 1) * 512],
                                           p5[:])
                        else:
                            nc.vector.tensor_copy(
                                otile[:, qh * 512:(qh + 1) * 512], p5[:])
                    eng = nc.sync if store_q[0] % 2 == 0 else nc.gpsimd
                    store_q[0] += 1
                    eng.dma_start(out[r0:r0 + 128, :], otile[:])

            N = len(TILES)
            stage_a(0, first=1)
            stage_a(1, first=2)
            stage_b(0)
            for k in range(2, N):
                stage_a(k)
                stage_b(k - 1)
                stage_c(k - 2)
            stage_b(N - 1)
            stage_c(N - 2)
            stage_c(N - 1)
    nc.compile()
    return nc


def _prep_shared(Wproj, Wdown, gamma, beta, Wl1, bl1, Wl2, bl2, Wup):
    f32 = np.float32
    wprojT = np.ascontiguousarray(Wproj.T).astype(f32, copy=False)
    wdcent = Wdown - Wdown.mean(axis=0, keepdims=True)
    wdc = np.ascontiguousarray(wdcent.T).astype(f32, copy=False)
    ones64 = np.full((D, D), 1.0 / D, dtype=f32)
    w1aug = np.empty((D + 1, D), dtype=f32)
    w1aug[:D] = (Wl1 * gamma[None, :]).T
    w1aug[D] = Wl1 @ beta + bl1
    w2aug = np.empty((D + 1, D), dtype=f32)
    w2aug[:D] = (Wl2 * gamma[None, :]).T
    w2aug[D] = Wl2 @ beta + bl2
    wupT = np.ascontiguousarray(Wup.T).astype(f32, copy=False)
    return dict(wprojT=wprojT, wdc=wdc, ones64=ones64,
                w1aug=w1aug, w2aug=w2aug, wupT=wupT)


def _ref_rows(X_rows, P):
    """numpy reference (up only, no residual) for a few token rows."""
    proj = X_rows @ P["wprojT"]                       # [n, Q]
    c = proj @ P["wdc"]                               # [n, D]
    var = (c * c).mean(axis=1, keepdims=True)
    z = c / np.sqrt(var + EPS)
    zaug = np.concatenate([z, np.ones((z.shape[0], 1), z.dtype)], axis=1)
    o1 = zaug @ P["w1aug"]
    gate = zaug @ P["w2aug"]
    act = o1 / (1.0 + np.exp(-o1)) * gate
    return act @ P["wupT"]


def kernel(clamp3_features, residual, Wproj, Wdown, gamma, beta,
           Wl1, bl1, Wl2, bl2, Wup):
    if "nc" not in _CACHE:
        _CACHE["nc"] = _build()
    nc = _CACHE["nc"]

    f32 = np.float32
    X = np.asarray(clamp3_features, dtype=f32).reshape(TOK, C)
    shared = _prep_shared(np.asarray(Wproj, f32), np.asarray(Wdown, f32),
                          np.asarray(gamma, f32), np.asarray(beta, f32),
                          np.asarray(Wl1, f32), np.asarray(bl1, f32),
                          np.asarray(Wl2, f32), np.asarray(bl2, f32),
                          np.asarray(Wup, f32))

    in_maps = []
    for cid in range(NCORES):
        lo, hi = cid * TPC, (cid + 1) * TPC
        in_maps.append({"xt": np.ascontiguousarray(X[lo:hi].T), **shared})

    # sampled self-check rows (2 per core) to catch transient bad executions
    rng = np.random.default_rng(12345)
    sample = np.sort(rng.choice(TPC, size=2, replace=False))
    Pd = {k: shared[k].astype(np.float64) for k in
          ("wprojT", "wdc", "w1aug", "w2aug", "wupT")}

    for attempt in range(3):
        res = bass_utils.run_bass_kernel_spmd(nc, in_maps,
                                              core_ids=list(range(NCORES)))
        outs = [res.results[cid]["out"] for cid in range(NCORES)]
        ok = True
        for cid in range(NCORES):
            rows = cid * TPC + sample
            ref = _ref_rows(X[rows].astype(np.float64), Pd)
            got = outs[cid][sample].astype(np.float64)
            err = np.abs(got - ref).max() / max(np.abs(ref).max(), 1e-30)
            if not np.isfinite(err) or err > 5e-3:
                ok = False
                break
        if ok:
            break

    up = np.concatenate(outs, axis=0).reshape(B, S, Q)
    return (np.asarray(residual, dtype=f32) + up).astype(np.float32, copy=False)
